# revision 17
# baseline (speedup 1.0000x reference)
"""TLGv4 block-sparse self-attention on 8 trn2 NeuronCores.

Sharding: tensor-parallel over the 8 KV groups (1 group = 4 Q heads + 1 K +
1 V head per core). Each core computes its group's QKV projection columns,
RoPE, block-sparse attention for its 4 Q heads, and a row-sharded partial of
the dense output projection. Host sums the 8 partials (+ b_dense).

Device dataflow per core (all matmuls fp32r = fp32 with 11-bit mantissa,
full PE rate at N=512):
  - qkvT[c, t] = wq_g @ hidden^T     (channels on partitions, tokens free)
  - RoPE on q/k rows via DVE with host-built cos/sin tables (q pre-scaled
    by 1/sqrt(D)); q repacked to qS[d, pair*512 + head*128 + t]
  - v^T transposed back to v[t, d] via PE, augmented with a ones column
  - per 128-query pair: scores^T[k, (h,q)] chunks via matmul, Exp on ACT
    (no max subtraction needed: |scores| < ~10), block-sparsity via memsets
    and a causal 0/1 mask multiply, PV accumulated over chunks; the ones
    column of V yields softmax denominators as ctx row 64; reciprocal is
    partition-broadcast via a DRAM bounce and multiplied in
  - dense partial: out[t, :] = ctx^T_g @ wd_g rows, streamed to DRAM
"""
import numpy as np
from contextlib import ExitStack

import concourse.bacc as bacc
import concourse.bass as bass
import concourse.mybir as mybir
import concourse.tile as tile
from concourse.bass_utils import run_bass_kernel_spmd

F32 = mybir.dt.float32
F32R = mybir.dt.float32r
F16 = mybir.dt.float16
AF = mybir.ActivationFunctionType

S = 2048
HID = 2048
D = 64
H_KV = 8
NQ = 4                      # q heads per kv group
GCOLS = (NQ + 2) * D        # 384 qkv columns per group
NPAIR = S // 128            # 16 pairs of 64-token blocks
SCALE = 1.0 / 8.0           # 1/sqrt(D)
ROPE_BASE = 10000.0
N_CORES = 8


def _r32r(x):
    u = np.ascontiguousarray(x, dtype=np.float32).view(np.uint32)
    u = (u.astype(np.uint64) + 0x800) & 0xFFFFF000
    return u.astype(np.uint32).view(np.float32).reshape(x.shape)


def _pair_chunks(i):
    """128-token k-chunks feeding query pair i (blocks 2i, 2i+1)."""
    chunks = list(range(max(0, i - 8), i + 1))
    if i >= 12:
        chunks = [3] + chunks
    return chunks


def _build_nc():
    nc = bacc.Bacc()

    ht = nc.declare_dram_parameter("ht", [HID, S], F16, isOutput=False)
    wq = nc.declare_dram_parameter("wq", [128, 16 * GCOLS], F16, isOutput=False)
    bq = nc.declare_dram_parameter("bq", [128, 3], F32, isOutput=False)
    wd = nc.declare_dram_parameter("wd", [128, 2 * HID], F16, isOutput=False)
    cosq = nc.declare_dram_parameter("cosq", [128, S], F16, isOutput=False)
    sinq = nc.declare_dram_parameter("sinq", [128, S], F16, isOutput=False)
    cosk = nc.declare_dram_parameter("cosk", [64, S], F16, isOutput=False)
    sink = nc.declare_dram_parameter("sink", [64, S], F16, isOutput=False)
    tri = nc.declare_dram_parameter("tri", [128, 128], F16, isOutput=False)
    ident = nc.declare_dram_parameter("ident", [128, 128], F16, isOutput=False)
    out = nc.declare_dram_parameter("out", [S, HID], F32, isOutput=True)

    scratch = nc.dram_tensor("scratch", [NPAIR, 512], F32)

    with tile.TileContext(nc) as tc, ExitStack() as ctx:
        consts = ctx.enter_context(tc.tile_pool(name="consts", bufs=1))
        persist = ctx.enter_context(tc.tile_pool(name="persist", bufs=1))

        wq_sb = consts.tile([128, 16 * GCOLS], F16)
        wd_sb = consts.tile([128, 2 * HID], F16)
        bq_sb = consts.tile([128, 3], F32)
        cosq_sb = consts.tile([128, S], F16)
        sinq_sb = consts.tile([128, S], F16)
        cosk_sb = consts.tile([64, S], F16)
        sink_sb = consts.tile([64, S], F16)
        tri_sb = consts.tile([128, 128], F16)
        id_sb = consts.tile([128, 128], F16)
        expb = consts.tile([128, 1], F32)
        nc.vector.memset(expb[:], -5.0)
        # small/early constants first; weight chunks split for fast start
        for t_, src in ((bq_sb, bq), (tri_sb, tri), (id_sb, ident)):
            nc.sync.dma_start(out=t_[:], in_=src[:, :])
        nc.sync.dma_start(out=wq_sb[:, 0:GCOLS], in_=wq[:, 0:GCOLS])

        # persistent activations
        qkv = [persist.tile([128, S], F16, tag=f"qkv{m}", name=f"qkv{m}")
               for m in range(3)]
        qS = persist.tile([64, NQ * S], F16)     # [d, pair*512 + h*128 + t]
        kT = persist.tile([64, S], F16)          # [d, t]
        v_sb = persist.tile([128, 16 * 66], F16)  # [t(128/chunk), chunk*66+d | ones]
        ctx_sb = persist.tile([128, 2 * S], F16)  # [(h%2)*64+d, (h//2)*2048 + t]

        v_r = v_sb[:].rearrange("p (c w) -> p c w", w=66)
        nc.vector.memset(v_r[:, :, 64:65], 1.0)
        nc.vector.memset(v_r[:, :, 65:66], 0.0)

        # ---- QKV + RoPE + V-transpose pipeline, per 512-token chunk ----
        with tc.tile_pool(name="hp", bufs=4) as hp, \
             tc.tile_pool(name="rope", bufs=2) as rp, \
             tc.tile_pool(name="psq", bufs=2, space="PSUM") as psq, \
             tc.tile_pool(name="pst", bufs=2, space="PSUM") as pst:
            for n in range(4):
                nsl = slice(n * 512, (n + 1) * 512)
                acc = [psq.tile([128, 512], F32, tag=f"a{m}", name=f"acc{m}")
                       for m in range(3)]
                for kq in range(4):
                    hch = hp.tile([128, 2048], F16)
                    src = ht[kq * 512:(kq + 1) * 512, nsl].rearrange(
                        "(c p) t -> p c t", p=128)
                    nc.sync.dma_start(out=hch[:].rearrange(
                        "p (c t) -> p c t", c=4), in_=src)
                    if n == 0:
                        for kn in range(kq * 4 + 1, kq * 4 + 5):
                            if kn < 16:
                                nc.sync.dma_start(
                                    out=wq_sb[:, kn * GCOLS:(kn + 1) * GCOLS],
                                    in_=wq[:, kn * GCOLS:(kn + 1) * GCOLS])
                    for kk in range(4):
                        kc = kq * 4 + kk
                        for mc in range(3):
                            nc.tensor.matmul(
                                acc[mc][:],
                                wq_sb[:, kc * GCOLS + mc * 128: kc * GCOLS + (mc + 1) * 128],
                                hch[:, kk * 512:(kk + 1) * 512],
                                start=(kc == 0), stop=(kc == 15))
                # later-phase constants ride the SWDGE queues (no contention
                # with the hidden-state stream on HWDGE)
                for t_, src in ((cosq_sb, cosq), (sinq_sb, sinq),
                                (cosk_sb, cosk), (sink_sb, sink)):
                    nc.sync.dma_start(out=t_[:, nsl], in_=src[:, nsl])
                nc.gpsimd.dma_start(out=wd_sb[:, n * 1024:(n + 1) * 1024],
                                    in_=wd[:, n * 1024:(n + 1) * 1024])
                for mc in range(3):
                    nc.vector.tensor_scalar_add(
                        qkv[mc][:, nsl], acc[mc][:], bq_sb[:, mc:mc + 1])
                # rope on this token chunk
                for ti in range(2):
                    qt = qkv[ti]
                    rot = rp.tile([128, 512], F16, tag="rot", name="rot")
                    for blk in range(4):
                        src = (blk ^ 1) * 32
                        nc.vector.tensor_copy(rot[blk * 32:(blk + 1) * 32, :],
                                              qt[src:src + 32, nsl])
                    tmp = rp.tile([128, 512], F16, tag="tmp", name="tmp")
                    nc.vector.tensor_mul(tmp[:], qt[:, nsl],
                                         cosq_sb[:, nsl])
                    nc.vector.tensor_mul(rot[:], rot[:], sinq_sb[:, nsl])
                    for half in range(2):  # head 2*ti + half
                        h = 2 * ti + half
                        dst = qS[:, n * 2048:(n + 1) * 2048].rearrange(
                            "p (pp hh t) -> p pp hh t", hh=NQ, t=128)[:, :, h, :]
                        nc.vector.tensor_add(
                            dst,
                            tmp[half * 64:(half + 1) * 64, :].rearrange(
                                "p (pp t) -> p pp t", t=128),
                            rot[half * 64:(half + 1) * 64, :].rearrange(
                                "p (pp t) -> p pp t", t=128))
                # k rope (qkv[2] rows 0:64), reusing rot/tmp slots
                rotk = rp.tile([128, 512], F16, tag="rot", name="rotk")
                nc.vector.tensor_copy(rotk[0:32, :], qkv[2][32:64, nsl])
                nc.vector.tensor_copy(rotk[32:64, :], qkv[2][0:32, nsl])
                tmpk = rp.tile([128, 512], F16, tag="tmp", name="tmpk")
                nc.vector.tensor_mul(tmpk[0:64, :], qkv[2][0:64, nsl],
                                     cosk_sb[:, nsl])
                nc.vector.tensor_mul(rotk[0:64, :], rotk[0:64, :], sink_sb[:, nsl])
                nc.vector.tensor_add(kT[:, nsl], tmpk[0:64, :], rotk[0:64, :])
                # v transpose for the 4 128-token chunks in this slice
                for cc in range(4):
                    c = 4 * n + cc
                    pt = pst.tile([128, 64], F16, name="pt")
                    nc.tensor.transpose(pt[:],
                                        qkv[2][64:128, c * 128:(c + 1) * 128],
                                        id_sb[64:128, 64:128])
                    nc.vector.tensor_copy(v_sb[:, c * 66:c * 66 + 64], pt[:])

        # ---- attention pairs + deferred dense ----
        # pair 11 first: it only needs rope(<=2) so it hides the rope(3) tail;
        # then big pairs descending; tiny pairs last under the dense backlog
        PAIR_ORDER = list(range(NPAIR))
        with tc.tile_pool(name="pss", bufs=4, space="PSUM") as pss, \
             tc.tile_pool(name="psc", bufs=2, space="PSUM") as psc, \
             tc.tile_pool(name="psd", bufs=2, space="PSUM") as psd, \
             tc.tile_pool(name="att", bufs=12) as att, \
             tc.tile_pool(name="ob", bufs=3) as ob, \
             tc.tile_pool(name="small", bufs=3) as small:

            def emit_dense(i):
                for nn in range(4):
                    dps = psd.tile([128, 512], F32, name="dps")
                    nc.tensor.matmul(dps[:],
                                     ctx_sb[:, i * 128:(i + 1) * 128],
                                     wd_sb[:, nn * 512:(nn + 1) * 512],
                                     start=True, stop=False)
                    nc.tensor.matmul(dps[:],
                                     ctx_sb[:, S + i * 128: S + (i + 1) * 128],
                                     wd_sb[:, HID + nn * 512: HID + (nn + 1) * 512],
                                     start=False, stop=True)
                    ost = ob.tile([128, 512], F32, name="ost")
                    nc.scalar.copy(ost[:], dps[:])
                    nc.scalar.dma_start(
                        out=out[i * 128:(i + 1) * 128, nn * 512:(nn + 1) * 512],
                        in_=ost[:])

            def emit_pair(i):
                chunks = _pair_chunks(i)
                ctx_ps = psc.tile([66, 512], F32, name="ctx_ps")
                exs = []
                # all score matmuls first: exp/masks complete in their shadow,
                # so the PV matmuls below never wait on ACT/DVE
                for c in chunks:
                    s_ps = pss.tile([128, 512], F32, name="s_ps")
                    nc.tensor.matmul(s_ps[:], kT[:, c * 128:(c + 1) * 128],
                                     qS[:, i * 512:(i + 1) * 512],
                                     start=True, stop=True)
                    ex = att.tile([128, 512], F16, tag="ex", name="ex")
                    nc.scalar.activation(ex[:], s_ps[:], AF.Exp, bias=expb[:])
                    exs.append(ex)
                    if c == i:  # diagonal: causal mask per head
                        for h in range(NQ):
                            nc.vector.tensor_mul(ex[:, h * 128:(h + 1) * 128],
                                                 ex[:, h * 128:(h + 1) * 128],
                                                 tri_sb[:])
                    elif i >= 8 and c == i - 8:
                        # first half-block invisible; second half only visible
                        # to the odd query block if it is a vertical block
                        nc.vector.memset(ex[0:64, :], 0.0)
                        if i % 4 != 3:
                            exr = ex[64:128, :].rearrange(
                                "p (hh t) -> p hh t", hh=NQ)
                            nc.vector.memset(exr[:, :, 64:128], 0.0)
                    elif i >= 12 and c == 3:
                        # vertical block 7 lives in chunk 3; block 6 invisible
                        nc.vector.memset(ex[0:64, :], 0.0)
                for idx, c in enumerate(chunks):
                    nc.tensor.matmul(ctx_ps[:], v_sb[:, c * 66:(c + 1) * 66],
                                     exs[idx][:], start=(idx == 0),
                                     stop=(idx == len(chunks) - 1))
                # denominators: scatter to [64,8], fast reciprocal, DRAM
                # bounce, partition-broadcast back as [64,512]
                den = small.tile([1, 512], F32, tag="den", name="den")
                nc.scalar.copy(den[:], ctx_ps[64:65, :])
                rec8 = small.tile([64, 8], F32, tag="rec8", name="rec8")
                nc.gpsimd.dma_start(out=rec8[:], in_=den[0:1, :].rearrange(
                    "o (p f) -> o p f", p=64))
                nc.vector.reciprocal(rec8[:], rec8[:])
                sc_row = scratch[i:i + 1, :]
                nc.gpsimd.dma_start(out=sc_row.rearrange("o (p f) -> o p f", p=64),
                                  in_=rec8[:])
                bcast = small.tile([64, 512], F32, tag="bc", name="bc")
                nc.gpsimd.dma_start(out=bcast[:], in_=bass.AP(
                    tensor=sc_row.tensor, offset=sc_row.offset,
                    ap=[[0, 64]] + sc_row.ap[1:]))
                for h in range(NQ):
                    nc.vector.tensor_mul(
                        ctx_sb[(h % 2) * 64:(h % 2) * 64 + 64,
                               (h // 2) * S + i * 128:(h // 2) * S + (i + 1) * 128],
                        ctx_ps[0:64, h * 128:(h + 1) * 128],
                        bcast[:, h * 128:(h + 1) * 128])

            for pidx, i in enumerate(PAIR_ORDER):
                emit_pair(i)
                if pidx >= 3:
                    emit_dense(PAIR_ORDER[pidx - 3])
            for i in PAIR_ORDER[-3:]:
                emit_dense(i)

    nc.finalize()
    return nc


_NC_CACHE = {}


def _get_nc():
    if "nc" not in _NC_CACHE:
        _NC_CACHE["nc"] = _build_nc()
    return _NC_CACHE["nc"]


def _host_inputs(hidden_states, w_qkv, b_qkv, w_dense):
    h = np.asarray(hidden_states, dtype=np.float32).reshape(S, HID)
    w_qkv = np.asarray(w_qkv, dtype=np.float32)
    b_qkv = np.asarray(b_qkv, dtype=np.float32)
    w_dense = np.asarray(w_dense, dtype=np.float32)

    ht = np.ascontiguousarray(h.T).astype(np.float16)

    inv = 1.0 / (ROPE_BASE ** (np.arange(0, D, 2, dtype=np.float32) / D))
    ang = np.arange(S, dtype=np.float32)[:, None] * inv[None, :]   # [S, 32]
    cosT = np.ascontiguousarray(np.cos(ang).T.astype(np.float32))  # [32, S]
    sinT = np.ascontiguousarray(np.sin(ang).T.astype(np.float32))
    cosq = (np.tile(cosT, (4, 1)) * SCALE).astype(np.float16)
    sinq = (np.concatenate([-sinT, sinT, -sinT, sinT], 0) * SCALE).astype(np.float16)
    cosk = np.tile(cosT, (2, 1)).astype(np.float16)
    sink = np.concatenate([-sinT, sinT], 0).astype(np.float16)

    tri = np.triu(np.ones((128, 128), np.float16))
    ident = np.eye(128, dtype=np.float16)

    in_maps = []
    for g in range(N_CORES):
        wqg = w_qkv[g * GCOLS:(g + 1) * GCOLS, :].T          # [HID, 384]
        wq_t = np.ascontiguousarray(
            wqg.reshape(16, 128, GCOLS).transpose(1, 0, 2).reshape(128, 16 * GCOLS)).astype(np.float16)
        bqg = np.ascontiguousarray(
            b_qkv[g * GCOLS:(g + 1) * GCOLS].reshape(3, 128).T)
        wdg = w_dense[:, g * NQ * D:(g + 1) * NQ * D].T      # [256, HID]
        wd_t = np.ascontiguousarray(
            wdg.reshape(2, 128, HID).transpose(1, 0, 2).reshape(128, 2 * HID)).astype(np.float16)
        in_maps.append({
            "ht": ht, "wq": wq_t, "bq": bqg, "wd": wd_t,
            "cosq": np.ascontiguousarray(cosq), "sinq": np.ascontiguousarray(sinq),
            "cosk": np.ascontiguousarray(cosk), "sink": np.ascontiguousarray(sink),
            "tri": tri, "ident": ident,
        })
    return in_maps


def run_device(hidden_states, w_qkv, b_qkv, w_dense, **run_kwargs):
    nc = _get_nc()
    in_maps = _host_inputs(hidden_states, w_qkv, b_qkv, w_dense)
    return run_bass_kernel_spmd(nc, in_maps, list(range(N_CORES)), **run_kwargs)


def kernel(hidden_states, w_qkv, b_qkv, w_dense, b_dense):
    res = run_device(hidden_states, w_qkv, b_qkv, w_dense)
    acc = np.zeros((S, HID), dtype=np.float32)
    for r in res.results:
        acc += r["out"]
    acc += np.asarray(b_dense, dtype=np.float32)[None, :]
    return acc.reshape(1, S, HID)
